# revision 9
# baseline (speedup 1.0000x reference)
"""DSAttention (de-stationary attention) TRN2 Bass kernel.

Computes, per (b, h):
    scores = (q @ k^T) * tau_b + delta_b          [L, S]
    scores = where(causal_mask, -1e9, scores)
    A = softmax(scale * scores)                    (no max-subtraction: logits O(10))
    out = A @ v                                    [L, D]

Strategy: batch*head parallel over 8 cores (4 (b,h) pairs per core).
Per (b,h), transposed-score space, j-outer over s-strips in two l-passes
(l in [0,1024) then [1024,2048)) so each kt_j / vp_j stationary is loaded
once per pass and causality is exact at 128-col granularity:
    XT_j[s, l] = sum_e KT[e, s] * QT[e, l] (+ aug row: 1.0 * scale*delta[s])
    p_j = exp(XT_j)      split across TWO engines:
          ACT: true exp LUT;  DVE: Schraudolph int16 bit-trick
          (bitcast fp16 approx of 2^(A*x+B), ~+/-3% sawtooth -- softmax
          scale-invariance cancels the mean, only the ripple remains)
    diag 128x128 blocks masked on gpsimd (affine_select, fill 0)
    OT[q][d, l] += V'_j[s, d] * p_j[s, l]  accumulated over j per quarter;
                   V' has a ones column -> row 64 = softmax denominator
Host divides numerator rows by the denominator row and un-transposes.

All matmul inputs are fp16 (measured: K=65 fp16 runs at the full
1 col/cycle PE rate, so the aug-row contraction is free and no pad rows /
memsets are needed). LDWEIGHTS fully overlaps back-to-back matmuls.
PE floor: 2*17408 cols/bh = 34816 cycles/bh at 2.4 GHz.
"""

import math
from collections import deque

import numpy as np

import bass_rust
import concourse.bass as bass
import concourse.mybir as mybir
import concourse.tile as tile
from concourse.bass_utils import run_bass_kernel_spmd

B, L, S, H, E, D = 2, 2048, 2048, 16, 64, 64
NCORES = 8
BH = B * H                      # 32 (b,h) pairs
BH_PER_CORE = BH // NCORES      # 4
SCALE = 1.0 / math.sqrt(E)
L_HALF = 1024

F32 = mybir.dt.float32
F16 = mybir.dt.float16
I16 = mybir.dt.int16

QK_COLS = 2 * L                 # qt | kt, each L wide, 65 rows (64 + aug)
VP_COLS = (S // 128) * (D + 1)  # 1040

# fp16 Schraudolph exp: e = bitcast_fp16(int16(rne(A16*x + B16)))
A16 = float(np.float32(1024.0 / np.log(2)))
B16 = 15291.0
LAG = 3                         # software pipeline depth, in exp-units


def _units_for_pass(p):
    """Strip groups sharing one exp instruction (packed into one PSUM tile)."""
    if p == 0:
        return [[0], [1], [2], [3], [4, 5], [6, 7]]
    return [[j] for j in range(12)] + [[12, 13], [14, 15]]


def _width(p, j):
    l0 = L_HALF * p
    return min(l0 + L_HALF, L) - max(l0, 128 * j)


def _assign_exp_engines():
    """Greedy static balance of exp units between ACT (true exp) and DVE
    (Schraudolph). Costs in ns per unit; loads seeded with the evac copies."""
    units = []
    for p in (0, 1):
        for u in _units_for_pass(p):
            units.append((p, tuple(u), sum(_width(p, j) for j in u)))
    loads = {"ACT": 1200.0, "DVE": 1408.0}
    out = {}
    for p, u, w in sorted(units, key=lambda t: -t[2]):
        ca, cd = 0.833 * w + 262, 1.042 * w + 170
        if loads["ACT"] + ca <= loads["DVE"] + cd:
            out[(p, u)] = "ACT"
            loads["ACT"] += ca
        else:
            out[(p, u)] = "DVE"
            loads["DVE"] += cd
    return out


_EXP_ENGINE = _assign_exp_engines()


class _SplitDrainTileContext(tile.TileContext):
    """This walrus build rejects instructions carrying more than one sem
    wait; the kernel-tail drain aggregates one wait per active processor.
    Split them across a chain of drains on the same engine."""

    def _drain_and_barrier(self, tick_clock, wait_clock):
        nc = self.nc
        drain_inst = nc.sync.drain()
        wait_clock.add_sem_waits(
            drain_inst.ins, bass_rust.ScopedClock({None: tick_clock.global_clock})
        )
        si = drain_inst.ins.sync_info
        waits = list(si.on_wait) if si is not None and si.on_wait else []
        if len(waits) > 1:
            si.on_wait = waits[:1]
            for w in waits[1:]:
                d2 = nc.sync.drain()
                d2.ins.sync_info = bass_rust.SyncInfo(on_wait=[w], on_update=[])
        nc.all_engine_barrier()
        popped = nc._tile_sem_poison_stack.pop()
        assert popped is self._sem_poison
        nc.clear_and_free_semaphores(list(self.sems.allocated().values()))
        nc.all_engine_barrier()


def _legalize_waits(nc, max_waits=1):
    """This walrus build rejects instructions with more than `max_waits`
    sem waits. Spill extras onto same-engine NoOps inserted just before
    the offending instruction (same-engine program order preserves the
    wait semantics)."""
    for f in nc.m.functions:
        for bb in f.blocks:
            insts = bb.instructions
            for idx in range(len(insts) - 1, -1, -1):
                inst = insts[idx]
                si = getattr(inst, "sync_info", None)
                if si is None or not si.on_wait:
                    continue
                ow = list(si.on_wait)
                sem = [w for w in ow if w.sync_type == "semaphore"]
                other = [w for w in ow if w.sync_type != "semaphore"]
                budget = max(0, max_waits - len(other))
                if len(sem) <= budget:
                    continue
                keep, spill = sem[:budget], sem[budget:]
                si.on_wait = other + keep
                for w in reversed(spill):
                    n = mybir.InstNoOp(name=f"W-{nc.next_id()}", ins=[], outs=[])
                    n.engine = inst.engine
                    n.sync_info = bass_rust.SyncInfo(on_wait=[w], on_update=[])
                    nc.register_instruction(n, overwrite=True)
                    insts.insert(idx, n)


def _build_program():
    nc = bass.Bass("TRN2", target_bir_lowering=False, debug=False)
    x_d = nc.declare_dram_parameter("x", [BH_PER_CORE, 65, QK_COLS], F16, isOutput=False)
    v_d = nc.declare_dram_parameter("v", [BH_PER_CORE, 128, VP_COLS], F16, isOutput=False)
    # output stays transposed: [bh, quarter, d, l_rel]; row d == D is the
    # softmax denominator; the host divides + un-transposes.
    o_d = nc.declare_dram_parameter("o", [BH_PER_CORE, 4, D + 1, 512], F32, isOutput=True)

    with _SplitDrainTileContext(nc) as tc:
        with (
            tc.tile_pool(name="xin", bufs=1) as in_pool,
            tc.tile_pool(name="p", bufs=LAG + 3) as p_pool,
            tc.tile_pool(name="osb", bufs=3) as osb_pool,
            tc.tile_pool(name="xt_ps", bufs=3, space="PSUM") as xt_pool,
            tc.tile_pool(name="ot_ps", bufs=2, space="PSUM") as ot_pool,
        ):
            xbs, vbs = [], []
            for i in range(BH_PER_CORE):
                xb = in_pool.tile([65, QK_COLS], F16, name=f"xb{i}", tag=f"xb{i}")
                vb = in_pool.tile([128, VP_COLS], F16, name=f"vb{i}", tag=f"vb{i}")
                xbs.append(xb)
                vbs.append(vb)
            # bh0's x rides two hardware DMA queues in priority-ordered
            # column-range pieces so the first exp-units' operands (first kt
            # strips + first qt half) land first; aggregate HBM bandwidth is
            # shared by all 8 cores and each queue sustains only ~40 GB/s,
            # so small high-priority pieces — not queue count — shorten the
            # preamble.
            for c0, c1 in [(L, L + 512), (L + 512, L + 1024), (1024, L)]:
                nc.sync.dma_start(out=xbs[0][:, c0:c1], in_=x_d[0, :, c0:c1])
            for c0, c1 in [(0, 512), (512, 1024), (L + 1024, 2 * L)]:
                nc.scalar.dma_start(out=xbs[0][:, c0:c1], in_=x_d[0, :, c0:c1])
            nc.gpsimd.dma_start(out=vbs[0], in_=v_d[0])
            for i in range(1, BH_PER_CORE):
                nc.sync.dma_start(out=xbs[i], in_=x_d[i])
                nc.sync.dma_start(out=vbs[i], in_=v_d[i])

            # pend: software pipeline of exp-units awaiting their AV matmuls.
            # Entries carry everything AV needs so the pipeline can run
            # across pass and bh boundaries without draining.
            pend = deque()
            ot = {}       # (i, q) -> psum tile, created lazily per pass
            nunit = 0

            def av_emit(item):
                i, pss, u, p_t, offs = item
                l0 = L_HALF * pss
                vb = vbs[i]
                for idx, j in enumerate(u):
                    ls = max(l0, 128 * j)
                    off = offs[idx]
                    for q in (2 * pss, 2 * pss + 1):
                        ql0 = 512 * q
                        a = max(ql0, ls)
                        if a >= ql0 + 512:
                            continue
                        nc.tensor.matmul(
                            ot[(i, q)][:, a - ql0:512],
                            lhsT=vb[:, 65 * j:65 * j + 65],
                            rhs=p_t[:, off + a - ls: off + ql0 + 512 - ls],
                            start=(j == 0), stop=(j == 4 * q + 3),
                        )
                    for q in (2 * pss, 2 * pss + 1):
                        if j == 4 * q + 3:
                            # quarter complete: evacuate + ship
                            osb = osb_pool.tile([D + 1, 512], F32, name=f"osb_{i}_{q}",
                                                tag="osb")
                            if i == BH_PER_CORE - 1 and q == 3:
                                # terminal quarter: split across both engines
                                # to shorten the kernel tail
                                nc.vector.tensor_copy(osb[:, 0:256],
                                                      ot[(i, q)][:, 0:256])
                                nc.scalar.activation(
                                    osb[:, 256:512], ot[(i, q)][:, 256:512],
                                    mybir.ActivationFunctionType.Copy)
                            elif q % 2 == 0:
                                nc.vector.tensor_copy(osb, ot[(i, q)])
                            else:
                                nc.scalar.activation(
                                    osb, ot[(i, q)],
                                    mybir.ActivationFunctionType.Copy)
                            nc.sync.dma_start(out=o_d[i, q], in_=osb)

            for i in range(BH_PER_CORE):
                xb = xbs[i]
                for pss in (0, 1):
                    l0 = L_HALF * pss
                    for q in (2 * pss, 2 * pss + 1):
                        ot[(i, q)] = ot_pool.tile([D + 1, 512], F32,
                                                  name=f"ot_{i}_{q}", tag="ot")
                    for u in _units_for_pass(pss):
                        widths = [_width(pss, j) for j in u]
                        offs = [sum(widths[:m]) for m in range(len(u))]
                        wu = sum(widths)
                        xt = xt_pool.tile([128, 1024], F32, name=f"xt{nunit}",
                                          tag="xt")
                        nunit += 1
                        for idx, j in enumerate(u):
                            ls = max(l0, 128 * j)
                            c0, c1 = offs[idx], offs[idx] + widths[idx]
                            # split at the PSUM bank boundary (tile col 512)
                            cuts = [c0, 512, c1] if c0 < 512 < c1 else [c0, c1]
                            for a, b in zip(cuts[:-1], cuts[1:]):
                                nc.tensor.matmul(
                                    xt[:, a:b],
                                    lhsT=xb[:, L + 128 * j: L + 128 * j + 128],
                                    rhs=xb[:, ls + a - c0: ls + b - c0],
                                    start=True, stop=True,
                                )
                        p_t = p_pool.tile([128, 1024], F16, name=f"p{nunit}",
                                          tag="p")
                        if _EXP_ENGINE[(pss, tuple(u))] == "ACT":
                            nc.scalar.activation(
                                p_t[:, 0:wu], xt[:, 0:wu],
                                mybir.ActivationFunctionType.Exp)
                        else:
                            nc.vector.tensor_scalar(
                                out=p_t[:, 0:wu].bitcast(I16), in0=xt[:, 0:wu],
                                scalar1=A16, scalar2=B16,
                                op0=mybir.AluOpType.mult,
                                op1=mybir.AluOpType.add)
                        for idx, j in enumerate(u):
                            if 128 * j >= l0:
                                # diagonal block: zero p where s > l
                                nc.gpsimd.affine_select(
                                    out=p_t[:, offs[idx]:offs[idx] + 128],
                                    in_=p_t[:, offs[idx]:offs[idx] + 128],
                                    compare_op=mybir.AluOpType.is_ge, fill=0.0,
                                    base=0, channel_multiplier=-1,
                                    pattern=[[1, 128]],
                                )
                        pend.append((i, pss, u, p_t, offs))
                        if len(pend) > LAG:
                            av_emit(pend.popleft())
            while pend:
                av_emit(pend.popleft())
    _legalize_waits(nc)
    return nc


_PROGRAM = None


def _get_program():
    global _PROGRAM
    if _PROGRAM is None:
        _PROGRAM = _build_program()
    return _PROGRAM


def _round_f32r(a):
    """Round fp32 to the f32r grid (13 low mantissa bits zeroed, RNE)."""
    b = a.astype(np.float32).view(np.uint32)
    r = (b + np.uint32(0x0FFF) + ((b >> np.uint32(13)) & np.uint32(1))) & ~np.uint32(0x1FFF)
    return r.view(np.float32)


def _prepare_inputs(q, k, v, tau, delta):
    """Pack full inputs into the per-core fp16 device layout."""
    qs = (q.astype(np.float64) * (SCALE * tau.astype(np.float64))[:, 0, None, None, None]).astype(np.float16)
    # [B,L,H,E] -> [BH, E, L]
    qt = np.ascontiguousarray(qs.transpose(0, 2, 3, 1).reshape(BH, E, L))
    kt = np.ascontiguousarray(k.astype(np.float16).transpose(0, 2, 3, 1).reshape(BH, E, S))
    # V' = [v, 1]: [BH, S, D+1] -> [BH, 128, 16*(D+1)]
    vt = v.astype(np.float16).transpose(0, 2, 1, 3).reshape(BH, S, D)
    vp = np.concatenate([vt, np.ones((BH, S, 1), np.float16)], axis=2)
    vp = np.ascontiguousarray(
        vp.reshape(BH, S // 128, 128, D + 1).transpose(0, 2, 1, 3).reshape(BH, 128, VP_COLS)
    )
    dsc = (SCALE * delta).astype(np.float16)  # [B, S]

    x = np.empty((BH, E + 1, QK_COLS), np.float16)
    x[:, 0:E, 0:L] = qt
    x[:, E, 0:L] = 1.0
    x[:, 0:E, L:2 * L] = kt
    x[:, E, L:2 * L] = np.repeat(dsc, H, axis=0)
    return x, vp


def _numpy_fallback(q, k, v, att_mask, tau, delta):
    out = np.empty((B, L, H, D), np.float32)
    mask = att_mask[:, 0]  # [B, L, S]
    for b in range(B):
        for h in range(H):
            s = (q[b, :, h, :] @ k[b, :, h, :].T) * tau[b, 0] + delta[b][None, :]
            s = np.where(mask[b], -1e9, s).astype(np.float32)
            s = SCALE * s
            s = s - s.max(axis=-1, keepdims=True)
            e = np.exp(s)
            a = e / e.sum(axis=-1, keepdims=True)
            out[b, :, h, :] = a @ v[b, :, h, :]
    return out


def kernel(q, k, v, att_mask, tau, delta):
    q = np.asarray(q, np.float32)
    k = np.asarray(k, np.float32)
    v = np.asarray(v, np.float32)
    tau = np.asarray(tau, np.float32)
    delta = np.asarray(delta, np.float32)
    att_mask = np.asarray(att_mask)

    causal = np.triu(np.ones((L, S), bool), k=1)
    if not all(np.array_equal(att_mask[b, 0], causal) for b in range(B)):
        return _numpy_fallback(q, k, v, att_mask, tau, delta)

    x, vp = _prepare_inputs(q, k, v, tau, delta)
    nc = _get_program()
    in_maps = [
        {
            "x": np.ascontiguousarray(x[c * BH_PER_CORE:(c + 1) * BH_PER_CORE]),
            "v": np.ascontiguousarray(vp[c * BH_PER_CORE:(c + 1) * BH_PER_CORE]),
        }
        for c in range(NCORES)
    ]
    res = run_bass_kernel_spmd(nc, in_maps, list(range(NCORES))).results

    out = np.empty((B, L, H, D), np.float32)
    for c in range(NCORES):
        o = res[c]["o"]  # [4, 4, D+1, 512]: raw numerators + denominator row
        norm = o[:, :, 0:D, :] / o[:, :, D:D + 1, :]
        for i in range(BH_PER_CORE):
            bh = c * BH_PER_CORE + i
            out[bh // H, :, bh % H, :] = norm[i].transpose(0, 2, 1).reshape(L, D)
    return out
